# revision 39
# baseline (speedup 1.0000x reference)
"""Trainium2 Bass kernel for batched channel attention — fp8 Gram-path version.

Reference (per batch b; B=8, A=2048 tokens, D=1024 channels):
    q = x @ Wq.T ; k = x @ Wk.T ; v = x @ Wv.T            # (A, D)
    q,k,v -> (D, A); q,k L2-normalized over the token axis
    attn = softmax((qn @ kn.T) * temperature, axis=-1)    # (D, D)
    out  = attn @ v_da ; y = out.T @ Wo.T                 # (A, D)

Sharding: pure data parallelism, batch b -> core b, no collectives.

Key structure (validated in numpy sim + CoreSim, rel err ~1.6e-3 vs 2e-2):
- All six GEMM-equivalents run in fp8 (float8e4) with DoubleRow perf mode
  (2 fp8 contraction rows per PE cell); operands live in [128, 2, N]
  pair-tiles, a DR matmul contracts 256 rows. Loops are ordered so
  consecutive matmuls share the stationary operand.
- Gram path: G = x8^T x8; Mq/Mk = G @ W{q,k}T; T[e,d] = Mk^T @ WqT gives
  the scores already transposed, so the exp eviction lands directly in the
  layout the out-matmul needs (no P transposes). Norms come from
  colsum(M .* WT) (diag of W G W^T), avoiding q/k materialization.
- Softmax here is nearly uniform (Snorm std ~0.04), so P=exp(S)~1 would be
  destroyed by fp8 quantization. We store P8s = (P-1)*64 in fp8 and carry
  the rank-1 "uniform" channel exactly: cv[a] = colsum_e v (host fp32) and
  cwo[f] = colsum_d Wo.T (host fp32) enter via a K=4 bf16 hi/lo matmul
  into the final y accumulation. out is stored as o8 = out*4*invden in
  fp8 (the cv*invden[d] common part rides the rank-1; invden deviates
  from 1/1024 by only ~0.1%, making the split essentially exact).
- Scalar chains (1/nq, 1/nk, 4/denom) run in [128, 8] column space:
  row -> PE transpose -> sqrt/reciprocal across lanes (a [1,1024]
  reciprocal on one DVE lane costs ~6.5us; the column form is ~60x less).
- Weight scales: all W.T shipped as 16*W.T in fp8; x unscaled fp8. Scale
  bookkeeping: G8=G/64, M8=G W.T/16, T=scale-1, v8=v, o8=out*4*invden,
  y = psum/4096.
"""

import numpy as np

B, A, D = 8, 2048, 1024
P = 128
NCH = 512
NT = D // P       # 8 tiles per 1024-dim axis
AT = A // P       # 16 a-tiles
FPAIR = 4         # 256-row contraction pair-tiles over a 1024 dim
APAIR = A // 256  # 8 a-pairs

_CACHE = {}


def _ensure_path():
    import importlib.util
    import sys
    if importlib.util.find_spec("concourse") is None:
        sys.path.insert(0, "/opt/trn_rl_repo")


def build_bass():
    _ensure_path()
    import concourse.bacc as bacc
    import concourse.mybir as mybir
    import concourse.tile as tile
    from concourse.masks import make_identity

    dt = mybir.dt
    F8 = dt.float8e4
    BF = dt.bfloat16
    F32 = dt.float32
    AF = mybir.ActivationFunctionType
    MULT = mybir.AluOpType.mult
    ADD = mybir.AluOpType.add
    SUB = mybir.AluOpType.subtract
    DR = mybir.MatmulPerfMode.DoubleRow

    nc = bacc.Bacc()

    xad_d = nc.declare_dram_parameter("xad", [A // 2, 2 * D], F8, isOutput=False)
    xt_d = nc.declare_dram_parameter("xt", [D // 2, 2 * A], F8, isOutput=False)
    wq_d = nc.declare_dram_parameter("wq", [D // 2, 2 * D], F8, isOutput=False)
    wk_d = nc.declare_dram_parameter("wk", [D // 2, 2 * D], F8, isOutput=False)
    wv_d = nc.declare_dram_parameter("wv", [D // 2, 2 * D], F8, isOutput=False)
    wo_d = nc.declare_dram_parameter("wo", [D // 2, 2 * D], F8, isOutput=False)
    uw_d = nc.declare_dram_parameter("uw", [4, A], BF, isOutput=False)
    wr_d = nc.declare_dram_parameter("wr", [4, D], BF, isOutput=False)
    tp_d = nc.declare_dram_parameter("temp", [1, 1], F32, isOutput=False)
    out_d = nc.declare_dram_parameter("out", [A, D], F32, isOutput=True)

    with tile.TileContext(nc) as tc:
        # ----------------------------- SBUF ------------------------------
        consts = tc.alloc_tile_pool(name="consts", bufs=1)
        ones8 = consts.tile([P, 2, P], F8, tag="ones8")
        nc.vector.memset(ones8[:], 1.0)
        ones_bf = consts.tile([1, P], BF, tag="ones_bf")
        nc.vector.memset(ones_bf[:], 1.0)
        one11 = consts.tile([1, 1], F32, tag="one11")
        nc.vector.memset(one11[:], 1.0)
        identf = consts.tile([P, P], F32, tag="identf")
        make_identity(nc, identf)
        t_sb = consts.tile([1, 1], F32, tag="t_sb")
        nc.sync.dma_start(t_sb[:], tp_d[:])
        inv_col = consts.tile([P, 2 * NT], F32, tag="inv_col")
        invd4c = consts.tile([P, NT], F32, tag="invd4c")
        uw_sb = consts.tile([4, A], BF, tag="uw_sb")
        wr_sb = consts.tile([4, D], BF, tag="wr_sb")
        nc.sync.dma_start(uw_sb[:], uw_d[:])
        nc.sync.dma_start(wr_sb[:], wr_d[:])

        misc = tc.alloc_tile_pool(name="misc", bufs=1)
        bcast_sb = misc.tile([P, D], F32, tag="bc", name="bcast_sb")
        bcq_row = misc.tile([1, D], BF, tag="bcq", name="bcq_row")

        w_p = tc.alloc_tile_pool(name="wp", bufs=FPAIR)

        def load_pairs(pool, dram, tag, cols, eng):
            ts = []
            for g in range(FPAIR):
                t = pool.tile([P, 2, cols], F8, tag=tag, name=f"{tag}{g}")
                eng.dma_start(t[:], dram[g * P:(g + 1) * P, :])
                ts.append(t)
            return ts

        g8_p = tc.alloc_tile_pool(name="g8", bufs=FPAIR)
        g8s = [g8_p.tile([P, 2, D], F8, tag="g8", name=f"g8_{i}")
               for i in range(FPAIR)]
        m_p = tc.alloc_tile_pool(name="m8", bufs=FPAIR)
        mq8 = [m_p.tile([P, 2, D], F8, tag="mq", name=f"mq{i}")
               for i in range(FPAIR)]
        mk8 = [m_p.tile([P, 2, D], F8, tag="mk", name=f"mk{i}")
               for i in range(FPAIR)]
        p8_p = tc.alloc_tile_pool(name="p8", bufs=FPAIR)
        p8s = [p8_p.tile([P, 2, D], F8, tag="p8", name=f"p8_{i}")
               for i in range(FPAIR)]
        v8_p = tc.alloc_tile_pool(name="v8", bufs=FPAIR)
        v8s = [v8_p.tile([P, 2, A], F8, tag="v8", name=f"v8_{i}")
               for i in range(FPAIR)]
        o8_p = tc.alloc_tile_pool(name="o8", bufs=FPAIR)
        o8s = [o8_p.tile([P, 2, A], F8, tag="o8", name=f"o8_{i}")
               for i in range(FPAIR)]

        # right side: xt (lives to phase 5), xad (dies after phase 1)
        xt_p = tc.alloc_tile_pool(name="xtp", bufs=FPAIR, side="right")
        xad_p = tc.alloc_tile_pool(name="xad", bufs=APAIR, side="right")
        dma_engs = [nc.sync, nc.gpsimd, nc.scalar]
        xads = []
        for u in range(APAIR):
            t = xad_p.tile([P, 2, D], F8, tag="xad", name=f"xad{u}")
            dma_engs[u % 3].dma_start(t[:], xad_d[u * P:(u + 1) * P, :])
            xads.append(t)
        wqs = load_pairs(w_p, wq_d, "wq", D, nc.sync)
        wks = load_pairs(w_p, wk_d, "wk", D, nc.gpsimd)
        xts = load_pairs(xt_p, xt_d, "xt", A, nc.scalar)
        wvs = load_pairs(w_p, wv_d, "wv", D, nc.sync)
        wos = load_pairs(w_p, wo_d, "wo", D, nc.scalar)

        # ------------- phase 1: G = x8^T x8 (a-contraction) --------------
        g_ps = tc.alloc_tile_pool(name="g_ps", bufs=4, space="PSUM")
        for gt in range(NT):
            gp = g_ps.tile([P, D], F32, tag="g")
            for u in range(APAIR):
                for c in range(2):
                    nc.tensor.matmul(
                        gp[:, c * NCH:(c + 1) * NCH],
                        xads[u][:, :, gt * P:(gt + 1) * P],
                        xads[u][:, :, c * NCH:(c + 1) * NCH],
                        start=(u == 0), stop=(u == APAIR - 1),
                        perf_mode=DR,
                    )
            dst = g8s[gt // 2][:, gt % 2:gt % 2 + 1, :]
            if gt % 2 == 0:
                nc.scalar.activation(dst, gp[:], AF.Copy, scale=1.0 / 64)
            else:
                nc.vector.tensor_scalar(
                    out=dst, in0=gp[:], scalar1=1.0 / 64, scalar2=None,
                    op0=MULT)
        g_ps.release()
        xad_p.release()

        # ------ phase 2: Mq/Mk = G @ W (g-contraction); E tiles on DVE ---
        s_p = tc.alloc_tile_pool(name="s_scr", bufs=2, side="right")
        e_p = tc.alloc_tile_pool(name="esc", bufs=FPAIR, side="right")
        e_ts = {}
        m_ps = tc.alloc_tile_pool(name="m_ps", bufs=4, space="PSUM")
        for ft in range(NT):
            mps = {}
            for w in range(2):
                mps[w] = m_ps.tile([P, D], F32, tag="m", name=f"m{ft}_{w}")
            for g in range(FPAIR):
                # same stationary operand for all 4 matmuls below
                for w, ws in enumerate((wqs, wks)):
                    for c in range(2):
                        nc.tensor.matmul(
                            mps[w][:, c * NCH:(c + 1) * NCH],
                            g8s[g][:, :, ft * P:(ft + 1) * P],
                            ws[g][:, :, c * NCH:(c + 1) * NCH],
                            start=(g == 0), stop=(g == FPAIR - 1),
                            perf_mode=DR,
                        )
            nc.scalar.activation(
                mq8[ft // 2][:, ft % 2:ft % 2 + 1, :], mps[0][:], AF.Copy,
                scale=1.0 / 4)
            nc.vector.tensor_scalar(
                out=mk8[ft // 2][:, ft % 2:ft % 2 + 1, :], in0=mps[1][:],
                scalar1=1.0 / 4, scalar2=None, op0=MULT)
            if ft % 2 == 1:
                fp = ft // 2
                for (m8, ws, tg, eng) in ((mq8, wqs, "eq", nc.gpsimd),
                                          (mk8, wks, "ek", nc.gpsimd)):
                    e_t = e_p.tile([P, 2, D], F8, tag=tg, name=f"{tg}{fp}")
                    eng.tensor_tensor(e_t[:], m8[fp][:], ws[fp][:], MULT)
                    e_ts[(tg, fp)] = e_t
        m_ps.release()

        # ---- phase 3: T et=0,1 to cover the norm chain; E colsums -------
        t_ps = tc.alloc_tile_pool(name="t_ps", bufs=2, space="PSUM")
        nrm_ps = tc.alloc_tile_pool(name="nrm_ps", bufs=2, space="PSUM",
                                    side="right")
        nq2 = nrm_ps.tile([P, D], F32, tag="nrm", name="nq2")
        nk2 = nrm_ps.tile([P, D], F32, tag="nrm", name="nk2")

        def t_mms(et):
            tp = t_ps.tile([P, D], F32, tag="t", name=f"t{et}")
            for g in range(FPAIR):
                for c in range(2):
                    nc.tensor.matmul(
                        tp[:, c * NCH:(c + 1) * NCH],
                        mk8[g][:, :, et * P:(et + 1) * P],
                        wqs[g][:, :, c * NCH:(c + 1) * NCH],
                        start=(g == 0), stop=(g == FPAIR - 1),
                        perf_mode=DR,
                    )
            return tp

        def t_evict(et, tp):
            s_scr = s_p.tile([P, D], F32, tag="s", name=f"s{et}")
            nc.vector.tensor_tensor(s_scr[:], tp[:], bcast_sb[:], MULT)
            p_scr = s_p.tile([P, D], F32, tag="p", name=f"pe{et}")
            nc.scalar.activation(
                p_scr[:], s_scr[:], AF.Exp,
                scale=inv_col[:, NT + et:NT + et + 1])
            nc.gpsimd.tensor_scalar(
                out=p8s[et // 2][:, et % 2:et % 2 + 1, :], in0=p_scr[:],
                scalar1=64.0, scalar2=64.0, op0=MULT, op1=SUB)

        tps = {}
        for et in range(2):
            tps[et] = t_mms(et)

        # E colsums -> nq2/nk2 (PE work that hides the chain latency)
        for fp in range(FPAIR):
            for (tg, ns) in (("eq", nq2), ("ek", nk2)):
                for c in range(2):
                    nc.tensor.matmul(
                        ns[:, c * NCH:(c + 1) * NCH],
                        ones8[:],
                        e_ts[(tg, fp)][:, :, c * NCH:(c + 1) * NCH],
                        start=(fp == 0), stop=(fp == FPAIR - 1),
                        perf_mode=DR,
                    )
        nq_row = misc.tile([1, D], F32, tag="srow", bufs=2, name="nq_row")
        nc.scalar.activation(nq_row[:], nq2[0:1, :], AF.Sqrt)
        nk_row = misc.tile([1, D], F32, tag="srow", bufs=2, name="nk_row")
        nc.scalar.activation(nk_row[:], nk2[0:1, :], AF.Sqrt)
        nrm_ps.release()

        # transposes: nq|nk rows -> [128, 16] columns, then 1/x across lanes
        bc_ps = tc.alloc_tile_pool(name="bc_ps", bufs=1, space="PSUM",
                                   side="right")
        ncol = bc_ps.tile([P, 2 * NT], F32, tag="x", name="ncol")
        for j in range(NT):
            nc.tensor.transpose(
                ncol[:, j:j + 1], nq_row[0:1, j * P:(j + 1) * P], one11[:])
            nc.tensor.transpose(
                ncol[:, NT + j:NT + j + 1],
                nk_row[0:1, j * P:(j + 1) * P], one11[:])
        nc.vector.reciprocal(inv_col[:], ncol[:])
        # back to a row (bf16, *temp) for the free-axis broadcast of 1/nq*t
        rq = bc_ps.tile([1, D], F32, tag="x", name="rq")
        for j in range(NT):
            nc.tensor.transpose(
                rq[0:1, j * P:(j + 1) * P], inv_col[:, j:j + 1], identf[:])
        nc.vector.tensor_scalar(
            out=bcq_row[:], in0=rq[:], scalar1=t_sb[0:1, 0:1], scalar2=None,
            op0=MULT)
        bcp = bc_ps.tile([P, D], F32, tag="x", name="bcp")
        for c in range(2):
            nc.tensor.matmul(
                bcp[:, c * NCH:(c + 1) * NCH],
                ones_bf[:],
                bcq_row[0:1, c * NCH:(c + 1) * NCH],
            )
        nc.vector.tensor_copy(bcast_sb[:], bcp[:])

        # ------- phase 4: T = Mk^T @ WqT -> exp -> P8s (f-contraction) ----
        t_evict(0, tps[0])
        t_evict(1, tps[1])
        for et in range(2, NT):
            t_evict(et, t_mms(et))
        bc_ps.release()
        e_p.release()

        t_ps.release()

        # ---------------- phase 5: denom mms, then v = WvT^T @ xT ---------
        dn_ps = tc.alloc_tile_pool(name="dn_ps", bufs=1, space="PSUM",
                                   side="right")
        dn = dn_ps.tile([P, D], F32, tag="dn", name="dn")
        for g in range(FPAIR):
            for c in range(2):
                nc.tensor.matmul(
                    dn[:, c * NCH:(c + 1) * NCH],
                    ones8[:],
                    p8s[g][:, :, c * NCH:(c + 1) * NCH],
                    start=(g == 0), stop=(g == FPAIR - 1),
                    perf_mode=DR,
                )
        dn_row = misc.tile([1, D], F32, tag="srow", bufs=2, name="dn_row")
        nc.vector.tensor_copy(dn_row[:], dn[0:1, :])

        v_ps = tc.alloc_tile_pool(name="v_ps", bufs=6, space="PSUM")
        v_engs = [nc.vector, nc.scalar]
        for dt_ in range(NT):
            vps = [v_ps.tile([P, NCH], F32, tag="v", name=f"v{dt_}_{c}")
                   for c in range(4)]
            for g in range(FPAIR):
                for c in range(4):
                    nc.tensor.matmul(
                        vps[c][:],
                        wvs[g][:, :, dt_ * P:(dt_ + 1) * P],
                        xts[g][:, :, c * NCH:(c + 1) * NCH],
                        start=(g == 0), stop=(g == FPAIR - 1),
                        perf_mode=DR,
                    )
            for c in range(4):
                dst = v8s[dt_ // 2][:, dt_ % 2:dt_ % 2 + 1,
                                    c * NCH:(c + 1) * NCH]
                eng = v_engs[(dt_ * 4 + c) % 2]
                if eng is nc.scalar:
                    nc.scalar.activation(dst, vps[c][:], AF.Copy,
                                         scale=1.0 / 16)
                else:
                    eng.tensor_scalar(out=dst, in0=vps[c][:],
                                      scalar1=1.0 / 16, scalar2=None,
                                      op0=MULT)
            if dt_ == 0:
                # 4/denom in column space, overlapped with v matmuls
                dcol = dn_ps.tile([P, NT], F32, tag="dn", name="dcol")
                for j in range(NT):
                    nc.tensor.transpose(
                        dcol[:, j:j + 1], dn_row[0:1, j * P:(j + 1) * P],
                        one11[:])
                dsc = misc.tile([P, NT], F32, tag="dsc", name="dsc")
                nc.vector.tensor_scalar(
                    out=dsc[:], in0=dcol[:], scalar1=1.0 / 256,
                    scalar2=256.0, op0=MULT, op1=ADD)
                nc.vector.reciprocal(invd4c[:], dsc[:])

        v_ps.release()
        dn_ps.release()

        # ---------------- phase 6: out = P^T @ v (e-contraction) ----------
        o_ps = tc.alloc_tile_pool(name="o_ps", bufs=8, space="PSUM")
        o_engs = [nc.vector, nc.scalar]
        for dt_ in range(NT):
            ops = [o_ps.tile([P, NCH], F32, tag="o", name=f"o{dt_}_{c}")
                   for c in range(4)]
            for g in range(FPAIR):
                for c in range(4):
                    nc.tensor.matmul(
                        ops[c][:],
                        p8s[g][:, :, dt_ * P:(dt_ + 1) * P],
                        v8s[g][:, :, c * NCH:(c + 1) * NCH],
                        start=(g == 0), stop=(g == FPAIR - 1),
                        perf_mode=DR,
                    )
            for c in range(4):
                dst = o8s[dt_ // 2][:, dt_ % 2:dt_ % 2 + 1,
                                    c * NCH:(c + 1) * NCH]
                eng = o_engs[(dt_ * 4 + c) % 2]
                if eng is nc.scalar:
                    nc.scalar.activation(dst, ops[c][:], AF.Copy,
                                         scale=invd4c[:, dt_:dt_ + 1])
                else:
                    eng.tensor_scalar(out=dst, in0=ops[c][:],
                                      scalar1=invd4c[:, dt_:dt_ + 1],
                                      scalar2=None, op0=MULT)
        o_ps.release()

        # -------- phase 7: y = o8^T @ WoT + rank-1 (d-contraction) --------
        y_p = tc.alloc_tile_pool(name="yp", bufs=2, side="right")
        y_ps = tc.alloc_tile_pool(name="y_ps", bufs=4, space="PSUM")
        for at in range(AT):
            yp = y_ps.tile([P, D], F32, tag="y")
            for g in range(FPAIR):
                for c in range(2):
                    nc.tensor.matmul(
                        yp[:, c * NCH:(c + 1) * NCH],
                        o8s[g][:, :, at * P:(at + 1) * P],
                        wos[g][:, :, c * NCH:(c + 1) * NCH],
                        start=(g == 0), stop=False,
                        perf_mode=DR,
                    )
            for c in range(2):
                nc.tensor.matmul(
                    yp[:, c * NCH:(c + 1) * NCH],
                    uw_sb[:, at * P:(at + 1) * P],
                    wr_sb[:, c * NCH:(c + 1) * NCH],
                    start=False, stop=True,
                )
            y_sb = y_p.tile([P, D], F32, tag="y", name=f"y{at}")
            if at % 2 == 0:
                nc.vector.tensor_scalar(
                    out=y_sb[:], in0=yp[:], scalar1=1.0 / 4096, scalar2=None,
                    op0=MULT)
            else:
                nc.scalar.activation(y_sb[:], yp[:], AF.Copy,
                                     scale=1.0 / 4096)
            nc.sync.dma_start(out_d[at * P:(at + 1) * P, :], y_sb[:])

        y_ps.release()
        y_p.release()
        s_p.release()
        xt_p.release()
        o8_p.release()
        v8_p.release()
        p8_p.release()
        m_p.release()
        g8_p.release()
        w_p.release()
        misc.release()
        consts.release()

    nc.compile()
    return nc


def _host_inputs(x, Wq, Wk, Wv, Wo, temperature):
    import ml_dtypes
    F8 = ml_dtypes.float8_e4m3
    BF = ml_dtypes.bfloat16

    def q8c(t):
        return np.clip(t, -240, 240).astype(F8)

    def pair_rows(m):  # (1024, C) -> (512, 2C): row g*128+p, col i*C+c
        c = m.shape[1]
        return np.ascontiguousarray(
            m.reshape(4, 2, 128, c).transpose(0, 2, 1, 3).reshape(512, 2 * c))

    Wq32, Wk32, Wv32, Wo32 = (np.asarray(w, np.float32)
                              for w in (Wq, Wk, Wv, Wo))
    wq8 = pair_rows(q8c(16 * Wq32.T))
    wk8 = pair_rows(q8c(16 * Wk32.T))
    wv8 = pair_rows(q8c(16 * Wv32.T))
    wo8 = pair_rows(q8c(16 * Wo32.T))
    cwo = Wo32.T.sum(axis=0).astype(np.float32)     # (D,)
    wh = cwo.astype(BF)
    wl = (cwo - wh.astype(np.float32)).astype(BF)
    wr = np.ascontiguousarray(np.stack([wh, wh, wl, wl]))   # [4, D]
    cwv = Wv32.sum(axis=0).astype(np.float32)       # (D,)

    in_maps = []
    for b in range(B):
        xb = np.asarray(x[b], np.float32)
        x8 = q8c(xb)                                # (A, D), quantized once
        xad = np.ascontiguousarray(
            x8.reshape(8, 2, 128, D).transpose(0, 2, 1, 3).reshape(A // 2, 2 * D))
        xt8 = pair_rows(np.ascontiguousarray(x8.T))
        cv4 = (xb @ cwv) * (4096.0 / 1024.0)        # (A,) pre-scaled
        uh = cv4.astype(BF)
        ul = (cv4 - uh.astype(np.float32)).astype(BF)
        uw = np.ascontiguousarray(np.stack([uh, ul, uh, ul]))  # [4, A]
        in_maps.append({
            "xad": xad,
            "xt": xt8,
            "wq": wq8,
            "wk": wk8,
            "wv": wv8,
            "wo": wo8,
            "uw": uw,
            "wr": wr,
            "temp": np.asarray(temperature[b]).reshape(1, 1).astype(np.float32),
        })
    return in_maps


def run(x, Wq, Wk, Wv, Wo, temperature, trace=False, tmpdir=None):
    """Run on the 8 NeuronCores; returns (out, BassKernelResults)."""
    _ensure_path()
    from concourse.bass_utils import run_bass_kernel_spmd

    if "nc" not in _CACHE:
        _CACHE["nc"] = build_bass()
    nc = _CACHE["nc"]
    in_maps = _host_inputs(x, Wq, Wk, Wv, Wo, temperature)
    res = run_bass_kernel_spmd(
        nc, in_maps, core_ids=list(range(B)), trace=trace, tmpdir=tmpdir
    )
    out = np.stack([np.asarray(res.results[b]["out"]) for b in range(B)])
    return out.astype(np.float32), res


def kernel(x, Wq, Wk, Wv, Wo, temperature):
    out, _ = run(x, Wq, Wk, Wv, Wo, temperature, trace=False)
    return out


# revision 40
# speedup vs baseline: 1.6921x; 1.6921x over previous
"""Trainium2 Bass kernel for batched channel attention — fp8 Gram-path version.

Reference (per batch b; B=8, A=2048 tokens, D=1024 channels):
    q = x @ Wq.T ; k = x @ Wk.T ; v = x @ Wv.T            # (A, D)
    q,k,v -> (D, A); q,k L2-normalized over the token axis
    attn = softmax((qn @ kn.T) * temperature, axis=-1)    # (D, D)
    out  = attn @ v_da ; y = out.T @ Wo.T                 # (A, D)

Sharding: pure data parallelism, batch b -> core b, no collectives.

Key structure (validated in numpy sim + CoreSim, rel err ~1.6e-3 vs 2e-2):
- All six GEMM-equivalents run in fp8 (float8e4) with DoubleRow perf mode
  (2 fp8 contraction rows per PE cell); operands live in [128, 2, N]
  pair-tiles, a DR matmul contracts 256 rows. Loops are ordered so
  consecutive matmuls share the stationary operand.
- Gram path: G = x8^T x8; Mq/Mk = G @ W{q,k}T; T[e,d] = Mk^T @ WqT gives
  the scores already transposed, so the exp eviction lands directly in the
  layout the out-matmul needs (no P transposes). Norms come from
  colsum(M .* WT) (diag of W G W^T), avoiding q/k materialization.
- Softmax here is nearly uniform (Snorm std ~0.04), so P=exp(S)~1 would be
  destroyed by fp8 quantization. We store P8s = (P-1)*64 in fp8 and carry
  the rank-1 "uniform" channel exactly: cv[a] = colsum_e v (host fp32) and
  cwo[f] = colsum_d Wo.T (host fp32) enter via a K=4 bf16 hi/lo matmul
  into the final y accumulation. out is stored as o8 = out*4*invden in
  fp8 (the cv*invden[d] common part rides the rank-1; invden deviates
  from 1/1024 by only ~0.1%, making the split essentially exact).
- Scalar chains (1/nq, 1/nk, 4/denom) run in [128, 8] column space:
  row -> PE transpose -> sqrt/reciprocal across lanes (a [1,1024]
  reciprocal on one DVE lane costs ~6.5us; the column form is ~60x less).
- Weight scales: all W.T shipped as 16*W.T in fp8; x unscaled fp8. Scale
  bookkeeping: G8=G/64, M8=G W.T/16, T=scale-1, v8=v, o8=out*4*invden,
  y = psum/4096.
"""

import numpy as np

B, A, D = 8, 2048, 1024
P = 128
NCH = 512
NT = D // P       # 8 tiles per 1024-dim axis
AT = A // P       # 16 a-tiles
FPAIR = 4         # 256-row contraction pair-tiles over a 1024 dim
APAIR = A // 256  # 8 a-pairs

_CACHE = {}


def _ensure_path():
    import importlib.util
    import sys
    if importlib.util.find_spec("concourse") is None:
        sys.path.insert(0, "/opt/trn_rl_repo")


def build_bass():
    _ensure_path()
    import concourse.bacc as bacc
    import concourse.mybir as mybir
    import concourse.tile as tile
    from concourse.masks import make_identity

    dt = mybir.dt
    F8 = dt.float8e4
    BF = dt.bfloat16
    F32 = dt.float32
    AF = mybir.ActivationFunctionType
    MULT = mybir.AluOpType.mult
    ADD = mybir.AluOpType.add
    SUB = mybir.AluOpType.subtract
    DR = mybir.MatmulPerfMode.DoubleRow

    nc = bacc.Bacc()

    xad_d = nc.declare_dram_parameter("xad", [A // 2, 2 * D], F8, isOutput=False)
    xt_d = nc.declare_dram_parameter("xt", [D // 2, 2 * A], F8, isOutput=False)
    wq_d = nc.declare_dram_parameter("wq", [D // 2, 2 * D], F8, isOutput=False)
    wk_d = nc.declare_dram_parameter("wk", [D // 2, 2 * D], F8, isOutput=False)
    wv_d = nc.declare_dram_parameter("wv", [D // 2, 2 * D], F8, isOutput=False)
    wo_d = nc.declare_dram_parameter("wo", [D // 2, 2 * D], F8, isOutput=False)
    uw_d = nc.declare_dram_parameter("uw", [4, A], BF, isOutput=False)
    wr_d = nc.declare_dram_parameter("wr", [4, D], BF, isOutput=False)
    tp_d = nc.declare_dram_parameter("temp", [1, 1], F32, isOutput=False)
    out_d = nc.declare_dram_parameter("out", [A, D], F32, isOutput=True)

    with tile.TileContext(nc) as tc:
        # ----------------------------- SBUF ------------------------------
        consts = tc.alloc_tile_pool(name="consts", bufs=1)
        ones8 = consts.tile([P, 2, P], F8, tag="ones8")
        nc.vector.memset(ones8[:], 1.0)
        ones_bf = consts.tile([1, P], BF, tag="ones_bf")
        nc.vector.memset(ones_bf[:], 1.0)
        one11 = consts.tile([1, 1], F32, tag="one11")
        nc.vector.memset(one11[:], 1.0)
        identf = consts.tile([P, P], F32, tag="identf")
        make_identity(nc, identf)
        t_sb = consts.tile([1, 1], F32, tag="t_sb")
        nc.sync.dma_start(t_sb[:], tp_d[:])
        inv_col = consts.tile([P, 2 * NT], F32, tag="inv_col")
        invd4c = consts.tile([P, NT], F32, tag="invd4c")
        uw_sb = consts.tile([4, A], BF, tag="uw_sb")
        wr_sb = consts.tile([4, D], BF, tag="wr_sb")
        nc.sync.dma_start(uw_sb[:], uw_d[:])
        nc.sync.dma_start(wr_sb[:], wr_d[:])

        misc = tc.alloc_tile_pool(name="misc", bufs=1)
        bcast_sb = misc.tile([P, D], F32, tag="bc", name="bcast_sb")
        bcq_row = misc.tile([1, D], BF, tag="bcq", name="bcq_row")

        w_p = tc.alloc_tile_pool(name="wp", bufs=FPAIR)

        def load_pairs(pool, dram, tag, cols, eng):
            ts = []
            for g in range(FPAIR):
                t = pool.tile([P, 2, cols], F8, tag=tag, name=f"{tag}{g}")
                eng.dma_start(t[:], dram[g * P:(g + 1) * P, :])
                ts.append(t)
            return ts

        g8_p = tc.alloc_tile_pool(name="g8", bufs=FPAIR)
        g8s = [g8_p.tile([P, 2, D], F8, tag="g8", name=f"g8_{i}")
               for i in range(FPAIR)]
        m_p = tc.alloc_tile_pool(name="m8", bufs=FPAIR)
        mq8 = [m_p.tile([P, 2, D], F8, tag="mq", name=f"mq{i}")
               for i in range(FPAIR)]
        mk8 = [m_p.tile([P, 2, D], F8, tag="mk", name=f"mk{i}")
               for i in range(FPAIR)]
        p8_p = tc.alloc_tile_pool(name="p8", bufs=FPAIR)
        p8s = [p8_p.tile([P, 2, D], F8, tag="p8", name=f"p8_{i}")
               for i in range(FPAIR)]
        v8_p = tc.alloc_tile_pool(name="v8", bufs=FPAIR)
        v8s = [v8_p.tile([P, 2, A], F8, tag="v8", name=f"v8_{i}")
               for i in range(FPAIR)]
        o8_p = tc.alloc_tile_pool(name="o8", bufs=FPAIR)
        o8s = [o8_p.tile([P, 2, A], F8, tag="o8", name=f"o8_{i}")
               for i in range(FPAIR)]

        # right side: xt (lives to phase 5), xad (dies after phase 1)
        xt_p = tc.alloc_tile_pool(name="xtp", bufs=FPAIR, side="right")
        xad_p = tc.alloc_tile_pool(name="xad", bufs=APAIR, side="right")
        dma_engs = [nc.sync, nc.gpsimd, nc.scalar]
        xads = []
        for u in range(APAIR):
            t = xad_p.tile([P, 2, D], F8, tag="xad", name=f"xad{u}")
            dma_engs[u % 3].dma_start(t[:], xad_d[u * P:(u + 1) * P, :])
            xads.append(t)
        wqs = load_pairs(w_p, wq_d, "wq", D, nc.sync)
        wks = load_pairs(w_p, wk_d, "wk", D, nc.gpsimd)
        xts = load_pairs(xt_p, xt_d, "xt", A, nc.scalar)
        wvs = load_pairs(w_p, wv_d, "wv", D, nc.sync)
        wos = load_pairs(w_p, wo_d, "wo", D, nc.scalar)

        # ------------- phase 1: G = x8^T x8 (a-contraction) --------------
        g_ps = tc.alloc_tile_pool(name="g_ps", bufs=4, space="PSUM")
        for gt in range(NT):
            gp = g_ps.tile([P, D], F32, tag="g")
            for c in range(2):
                for u in range(APAIR):
                    nc.tensor.matmul(
                        gp[:, c * NCH:(c + 1) * NCH],
                        xads[u][:, :, gt * P:(gt + 1) * P],
                        xads[u][:, :, c * NCH:(c + 1) * NCH],
                        start=(u == 0), stop=(u == APAIR - 1),
                        perf_mode=DR,
                    )
            dst = g8s[gt // 2][:, gt % 2:gt % 2 + 1, :]
            if gt % 2 == 0:
                nc.scalar.activation(dst, gp[:], AF.Copy, scale=1.0 / 64)
            else:
                nc.vector.tensor_scalar(
                    out=dst, in0=gp[:], scalar1=1.0 / 64, scalar2=None,
                    op0=MULT)
        g_ps.release()
        xad_p.release()

        # ------ phase 2: Mq/Mk = G @ W (g-contraction); E tiles on DVE ---
        s_p = tc.alloc_tile_pool(name="s_scr", bufs=2, side="right")
        e_p = tc.alloc_tile_pool(name="esc", bufs=FPAIR, side="right")
        e_ts = {}
        m_ps = tc.alloc_tile_pool(name="m_ps", bufs=4, space="PSUM")
        for ft in range(NT):
            mps = {}
            for w in range(2):
                mps[w] = m_ps.tile([P, D], F32, tag="m", name=f"m{ft}_{w}")
            for w, ws in enumerate((wqs, wks)):
                for c in range(2):
                    for g in range(FPAIR):
                        nc.tensor.matmul(
                            mps[w][:, c * NCH:(c + 1) * NCH],
                            g8s[g][:, :, ft * P:(ft + 1) * P],
                            ws[g][:, :, c * NCH:(c + 1) * NCH],
                            start=(g == 0), stop=(g == FPAIR - 1),
                            perf_mode=DR,
                        )
            nc.scalar.activation(
                mq8[ft // 2][:, ft % 2:ft % 2 + 1, :], mps[0][:], AF.Copy,
                scale=1.0 / 4)
            nc.vector.tensor_scalar(
                out=mk8[ft // 2][:, ft % 2:ft % 2 + 1, :], in0=mps[1][:],
                scalar1=1.0 / 4, scalar2=None, op0=MULT)
            if ft % 2 == 1:
                fp = ft // 2
                for (m8, ws, tg, eng) in ((mq8, wqs, "eq", nc.vector),
                                          (mk8, wks, "ek", nc.vector)):
                    e_t = e_p.tile([P, 2, D], F8, tag=tg, name=f"{tg}{fp}")
                    eng.tensor_tensor(e_t[:], m8[fp][:], ws[fp][:], MULT)
                    e_ts[(tg, fp)] = e_t
        m_ps.release()

        # ---- phase 3: T et=0,1 to cover the norm chain; E colsums -------
        t_ps = tc.alloc_tile_pool(name="t_ps", bufs=2, space="PSUM")
        nrm_ps = tc.alloc_tile_pool(name="nrm_ps", bufs=2, space="PSUM",
                                    side="right")
        nq2 = nrm_ps.tile([P, D], F32, tag="nrm", name="nq2")
        nk2 = nrm_ps.tile([P, D], F32, tag="nrm", name="nk2")

        def t_mms(et):
            tp = t_ps.tile([P, D], F32, tag="t", name=f"t{et}")
            for c in range(2):
                for g in range(FPAIR):
                    nc.tensor.matmul(
                        tp[:, c * NCH:(c + 1) * NCH],
                        mk8[g][:, :, et * P:(et + 1) * P],
                        wqs[g][:, :, c * NCH:(c + 1) * NCH],
                        start=(g == 0), stop=(g == FPAIR - 1),
                        perf_mode=DR,
                    )
            return tp

        def t_evict(et, tp):
            s_scr = s_p.tile([P, D], F32, tag="s", name=f"s{et}")
            nc.vector.tensor_tensor(s_scr[:], tp[:], bcast_sb[:], MULT)
            p_scr = s_p.tile([P, D], F32, tag="p", name=f"pe{et}")
            nc.scalar.activation(
                p_scr[:], s_scr[:], AF.Exp,
                scale=inv_col[:, NT + et:NT + et + 1])
            nc.vector.tensor_scalar(
                out=p8s[et // 2][:, et % 2:et % 2 + 1, :], in0=p_scr[:],
                scalar1=64.0, scalar2=64.0, op0=MULT, op1=SUB)

        tps = {}
        for et in range(2):
            tps[et] = t_mms(et)

        # E colsums -> nq2/nk2 (PE work that hides the chain latency)
        for fp in range(FPAIR):
            for (tg, ns) in (("eq", nq2), ("ek", nk2)):
                for c in range(2):
                    nc.tensor.matmul(
                        ns[:, c * NCH:(c + 1) * NCH],
                        ones8[:],
                        e_ts[(tg, fp)][:, :, c * NCH:(c + 1) * NCH],
                        start=(fp == 0), stop=(fp == FPAIR - 1),
                        perf_mode=DR,
                    )
        nq_row = misc.tile([1, D], F32, tag="srow", bufs=2, name="nq_row")
        nc.scalar.activation(nq_row[:], nq2[0:1, :], AF.Sqrt)
        nk_row = misc.tile([1, D], F32, tag="srow", bufs=2, name="nk_row")
        nc.scalar.activation(nk_row[:], nk2[0:1, :], AF.Sqrt)
        nrm_ps.release()

        # transposes: nq|nk rows -> [128, 16] columns, then 1/x across lanes
        bc_ps = tc.alloc_tile_pool(name="bc_ps", bufs=1, space="PSUM",
                                   side="right")
        ncol = bc_ps.tile([P, 2 * NT], F32, tag="x", name="ncol")
        for j in range(NT):
            nc.tensor.transpose(
                ncol[:, j:j + 1], nq_row[0:1, j * P:(j + 1) * P], one11[:])
            nc.tensor.transpose(
                ncol[:, NT + j:NT + j + 1],
                nk_row[0:1, j * P:(j + 1) * P], one11[:])
        nc.vector.reciprocal(inv_col[:], ncol[:])
        # back to a row (bf16, *temp) for the free-axis broadcast of 1/nq*t
        rq = bc_ps.tile([1, D], F32, tag="x", name="rq")
        for j in range(NT):
            nc.tensor.transpose(
                rq[0:1, j * P:(j + 1) * P], inv_col[:, j:j + 1], identf[:])
        nc.vector.tensor_scalar(
            out=bcq_row[:], in0=rq[:], scalar1=t_sb[0:1, 0:1], scalar2=None,
            op0=MULT)
        bcp = bc_ps.tile([P, D], F32, tag="x", name="bcp")
        for c in range(2):
            nc.tensor.matmul(
                bcp[:, c * NCH:(c + 1) * NCH],
                ones_bf[:],
                bcq_row[0:1, c * NCH:(c + 1) * NCH],
            )
        nc.vector.tensor_copy(bcast_sb[:], bcp[:])

        # ------- phase 4: T = Mk^T @ WqT -> exp -> P8s (f-contraction) ----
        t_evict(0, tps[0])
        t_evict(1, tps[1])
        for et in range(2, NT):
            t_evict(et, t_mms(et))
        bc_ps.release()
        e_p.release()

        t_ps.release()

        # ---------------- phase 5: denom mms, then v = WvT^T @ xT ---------
        dn_ps = tc.alloc_tile_pool(name="dn_ps", bufs=1, space="PSUM",
                                   side="right")
        dn = dn_ps.tile([P, D], F32, tag="dn", name="dn")
        for c in range(2):
            for g in range(FPAIR):
                nc.tensor.matmul(
                    dn[:, c * NCH:(c + 1) * NCH],
                    ones8[:],
                    p8s[g][:, :, c * NCH:(c + 1) * NCH],
                    start=(g == 0), stop=(g == FPAIR - 1),
                    perf_mode=DR,
                )
        dn_row = misc.tile([1, D], F32, tag="srow", bufs=2, name="dn_row")
        nc.vector.tensor_copy(dn_row[:], dn[0:1, :])

        v_ps = tc.alloc_tile_pool(name="v_ps", bufs=6, space="PSUM")
        v_engs = [nc.vector, nc.scalar]
        for dt_ in range(NT):
            vps = [v_ps.tile([P, NCH], F32, tag="v", name=f"v{dt_}_{c}")
                   for c in range(4)]
            for c in range(4):
                for g in range(FPAIR):
                    nc.tensor.matmul(
                        vps[c][:],
                        wvs[g][:, :, dt_ * P:(dt_ + 1) * P],
                        xts[g][:, :, c * NCH:(c + 1) * NCH],
                        start=(g == 0), stop=(g == FPAIR - 1),
                        perf_mode=DR,
                    )
            for c in range(4):
                dst = v8s[dt_ // 2][:, dt_ % 2:dt_ % 2 + 1,
                                    c * NCH:(c + 1) * NCH]
                eng = v_engs[(dt_ * 4 + c) % 2]
                if eng is nc.scalar:
                    nc.scalar.activation(dst, vps[c][:], AF.Copy,
                                         scale=1.0 / 16)
                else:
                    eng.tensor_scalar(out=dst, in0=vps[c][:],
                                      scalar1=1.0 / 16, scalar2=None,
                                      op0=MULT)
            if dt_ == 0:
                # 4/denom in column space, overlapped with v matmuls
                dcol = dn_ps.tile([P, NT], F32, tag="dn", name="dcol")
                for j in range(NT):
                    nc.tensor.transpose(
                        dcol[:, j:j + 1], dn_row[0:1, j * P:(j + 1) * P],
                        one11[:])
                dsc = misc.tile([P, NT], F32, tag="dsc", name="dsc")
                nc.vector.tensor_scalar(
                    out=dsc[:], in0=dcol[:], scalar1=1.0 / 256,
                    scalar2=256.0, op0=MULT, op1=ADD)
                nc.vector.reciprocal(invd4c[:], dsc[:])

        v_ps.release()
        dn_ps.release()

        # ---------------- phase 6: out = P^T @ v (e-contraction) ----------
        o_ps = tc.alloc_tile_pool(name="o_ps", bufs=8, space="PSUM")
        o_engs = [nc.vector, nc.scalar]
        for dt_ in range(NT):
            ops = [o_ps.tile([P, NCH], F32, tag="o", name=f"o{dt_}_{c}")
                   for c in range(4)]
            for c in range(4):
                for g in range(FPAIR):
                    nc.tensor.matmul(
                        ops[c][:],
                        p8s[g][:, :, dt_ * P:(dt_ + 1) * P],
                        v8s[g][:, :, c * NCH:(c + 1) * NCH],
                        start=(g == 0), stop=(g == FPAIR - 1),
                        perf_mode=DR,
                    )
            for c in range(4):
                dst = o8s[dt_ // 2][:, dt_ % 2:dt_ % 2 + 1,
                                    c * NCH:(c + 1) * NCH]
                eng = o_engs[(dt_ * 4 + c) % 2]
                if eng is nc.scalar:
                    nc.scalar.activation(dst, ops[c][:], AF.Copy,
                                         scale=invd4c[:, dt_:dt_ + 1])
                else:
                    eng.tensor_scalar(out=dst, in0=ops[c][:],
                                      scalar1=invd4c[:, dt_:dt_ + 1],
                                      scalar2=None, op0=MULT)
        o_ps.release()

        # -------- phase 7: y = o8^T @ WoT + rank-1 (d-contraction) --------
        y_p = tc.alloc_tile_pool(name="yp", bufs=2, side="right")
        y_ps = tc.alloc_tile_pool(name="y_ps", bufs=4, space="PSUM")
        for at in range(AT):
            yp = y_ps.tile([P, D], F32, tag="y")
            for c in range(2):
                for g in range(FPAIR):
                    nc.tensor.matmul(
                        yp[:, c * NCH:(c + 1) * NCH],
                        o8s[g][:, :, at * P:(at + 1) * P],
                        wos[g][:, :, c * NCH:(c + 1) * NCH],
                        start=(g == 0), stop=False,
                        perf_mode=DR,
                    )
                nc.tensor.matmul(
                    yp[:, c * NCH:(c + 1) * NCH],
                    uw_sb[:, at * P:(at + 1) * P],
                    wr_sb[:, c * NCH:(c + 1) * NCH],
                    start=False, stop=True,
                )
            y_sb = y_p.tile([P, D], F32, tag="y", name=f"y{at}")
            if at % 2 == 0:
                nc.vector.tensor_scalar(
                    out=y_sb[:], in0=yp[:], scalar1=1.0 / 4096, scalar2=None,
                    op0=MULT)
            else:
                nc.scalar.activation(y_sb[:], yp[:], AF.Copy,
                                     scale=1.0 / 4096)
            nc.sync.dma_start(out_d[at * P:(at + 1) * P, :], y_sb[:])

        y_ps.release()
        y_p.release()
        s_p.release()
        xt_p.release()
        o8_p.release()
        v8_p.release()
        p8_p.release()
        m_p.release()
        g8_p.release()
        w_p.release()
        misc.release()
        consts.release()

    nc.compile()
    return nc


def _host_inputs(x, Wq, Wk, Wv, Wo, temperature):
    import ml_dtypes
    F8 = ml_dtypes.float8_e4m3
    BF = ml_dtypes.bfloat16

    def q8c(t):
        return np.clip(t, -240, 240).astype(F8)

    def pair_rows(m):  # (1024, C) -> (512, 2C): row g*128+p, col i*C+c
        c = m.shape[1]
        return np.ascontiguousarray(
            m.reshape(4, 2, 128, c).transpose(0, 2, 1, 3).reshape(512, 2 * c))

    Wq32, Wk32, Wv32, Wo32 = (np.asarray(w, np.float32)
                              for w in (Wq, Wk, Wv, Wo))
    wq8 = pair_rows(q8c(16 * Wq32.T))
    wk8 = pair_rows(q8c(16 * Wk32.T))
    wv8 = pair_rows(q8c(16 * Wv32.T))
    wo8 = pair_rows(q8c(16 * Wo32.T))
    cwo = Wo32.T.sum(axis=0).astype(np.float32)     # (D,)
    wh = cwo.astype(BF)
    wl = (cwo - wh.astype(np.float32)).astype(BF)
    wr = np.ascontiguousarray(np.stack([wh, wh, wl, wl]))   # [4, D]
    cwv = Wv32.sum(axis=0).astype(np.float32)       # (D,)

    in_maps = []
    for b in range(B):
        xb = np.asarray(x[b], np.float32)
        x8 = q8c(xb)                                # (A, D), quantized once
        xad = np.ascontiguousarray(
            x8.reshape(8, 2, 128, D).transpose(0, 2, 1, 3).reshape(A // 2, 2 * D))
        xt8 = pair_rows(np.ascontiguousarray(x8.T))
        cv4 = (xb @ cwv) * (4096.0 / 1024.0)        # (A,) pre-scaled
        uh = cv4.astype(BF)
        ul = (cv4 - uh.astype(np.float32)).astype(BF)
        uw = np.ascontiguousarray(np.stack([uh, ul, uh, ul]))  # [4, A]
        in_maps.append({
            "xad": xad,
            "xt": xt8,
            "wq": wq8,
            "wk": wk8,
            "wv": wv8,
            "wo": wo8,
            "uw": uw,
            "wr": wr,
            "temp": np.asarray(temperature[b]).reshape(1, 1).astype(np.float32),
        })
    return in_maps


def run(x, Wq, Wk, Wv, Wo, temperature, trace=False, tmpdir=None):
    """Run on the 8 NeuronCores; returns (out, BassKernelResults)."""
    _ensure_path()
    from concourse.bass_utils import run_bass_kernel_spmd

    if "nc" not in _CACHE:
        _CACHE["nc"] = build_bass()
    nc = _CACHE["nc"]
    in_maps = _host_inputs(x, Wq, Wk, Wv, Wo, temperature)
    res = run_bass_kernel_spmd(
        nc, in_maps, core_ids=list(range(B)), trace=trace, tmpdir=tmpdir
    )
    out = np.stack([np.asarray(res.results[b]["out"]) for b in range(B)])
    return out.astype(np.float32), res


def kernel(x, Wq, Wk, Wv, Wo, temperature):
    out, _ = run(x, Wq, Wk, Wv, Wo, temperature, trace=False)
    return out
